# revision 20
# baseline (speedup 1.0000x reference)
"""Trainium2 Bass kernel for nn_ContinuousLearningLayer.

Computes, for flattened input x[N=1024] and flattened weights w[M=262144]:
    out[n, m] = max_{j in [m-25, m+25] ∩ [0,M)} 1{ |x[n] - w[j]| < 0.1 }
i.e. a binary mask |x-w|<0.1 dilated by a width-51 window along the weight
axis.  Output is [1024, 262144] fp32 of {0.0, 1.0} (~1 GB) — memory bound.

Strategy (8 NeuronCores, no communication):
  * Shard the M (weight) axis: core c owns m in [c*32768, (c+1)*32768).
    Each core gets its w slice padded with +-25 halo; out-of-range
    positions are filled with a 1e9 sentinel (never a hit), so edge
    windows need no special casing.
  * On-chip layout is transposed: mask tiles are [128 j x 1024 n]
    (weight index on partitions, ALL batch rows on the free dim); the
    windowed OR becomes a banded ones-matrix matmul on the tensor
    engine:
        count[n, m] = sum_j mask[j, n] * T[j, m],   T[j,m]=1{0<=j-m<=50}
    windowed-max of a 0/1 mask == (windowed count > 0).

Engine economics (per core, measured):
  * out-DMA: 128 MiB; SWDGE bf16->fp32 cast-DMA sustains ~387 GB/s
    -> ~347 us active.  (HWDGE fp32 was 358 GB/s / ~378 us.)
  * drains (PSUM->SBUF threshold): PSUM fp32 reads are capped at
    1 elem/cyc (single read port) -> ~1.92us/2048 on ACT (Sign),
    ~2.29us on DVE (is_gt).  ~130 full-tile equivalents total.
  * mask gen, two forms:
      abs-form:  ACT  ad = Abs(1.0*x + (-w))      1141 ns
                 DVE  mk = is_lt(ad, 0.1) bf16     688 ns
      pair-form: DVE  mk+ = (x + (-w)) is_lt  0.1  (fused 2-op, 688 ns)
                 DVE  mk- = (x + (-w)) is_le -0.1  (fused 2-op, 688 ns)
        and count += T*mk+ + (-T)*mk- on the PE: 1{t<0.1}-1{t<=-0.1}
        == 1{|t|<0.1} exactly (and 1e9 sentinels cancel to 0).
    The pair-form removes ACT work at the cost of doubled PE columns
    for that tau; PE has ~100us headroom, so ~1/3 of taus use it.
  * GPSIMD compute is useless here: TensorScalar ucode ~19us/tile,
    TensorTensor ISA-rejected on Pool.  GPSIMD only issues the
    cast-DMAs (SWDGE).
  * psum groups ramp [1,1,2] + 4s + [2,1,1] so the first output DMA
    issues at ~10 us and the drain tail is short.
"""

import os

import numpy as np
import ml_dtypes

import concourse.bass as bass
import concourse.bacc as bacc
import concourse.tile as tile
from concourse import mybir
from concourse.bass_utils import run_bass_kernel_spmd

# ---- problem constants (hardcoded; kernel.py must be self-contained) ----
N = 1024           # flattened input length  (2*512)
M = 262144         # flattened weight length (512*512)
NCORES = 8
MS = M // NCORES   # 32768 weight columns per core
PAD = 25           # window radius (width 51)
WIN = 51
JT = (MS + 2 * PAD + 127) // 128   # 257 j-tiles of 128 per core
JPAD = JT * 128                    # 32896 padded j range per core
MB = MS // 512                     # 64 psum banks of m per core
NB = N // 128                      # 8 n-blocks of 128
THRESH = np.float32(0.1)
BIG = np.float32(1.0e9)            # sentinel weight: never within 0.1 of any input

F32 = mybir.dt.float32
BF16 = mybir.dt.bfloat16

# T matrices: for output bank m-range [0,512), contributions come from
# j-tiles delta=0..4; T_delta[jl, mf] = 1 iff 0 <= (128*delta + jl) - mf <= 50.
# Only the nonzero mf-stripe of each is materialized:
#   delta:   mf offset   width
#     0         0         128
#     1        78         178
#     2       206         178
#     3       334         178
#     4       462          50
T_OFFS = [0, 78, 206, 334, 462]
T_WIDTHS = [128, 178, 178, 178, 50]
T_COLOFF = np.cumsum([0] + T_WIDTHS).tolist()  # offsets inside packed T tensor
T_TOTAL = sum(T_WIDTHS)  # 712; negated copy lives at columns [712, 1424)

# ---- tuning knobs -------------------------------------------------------
# percent of drains on ACT as Sign(count) (Sign(0)=0, Sign(pos)=1, exact);
# the rest on DVE as is_gt(count, 0)
TH_ACT_PCT = int(os.environ.get("CLK_TH_ACT_PCT", "32"))
# every k-th tau uses the pair-form mask (0 = never)
PM_EVERY = int(os.environ.get("CLK_PM_EVERY", "0"))
# 1: thresholds write bf16 {0,1} and the out-DMA casts bf16->fp32 in the
# SDMA datapath (SWDGE)
CAST_DMA = int(os.environ.get("CLK_CAST_DMA", "1"))
# 1: alternate out-DMAs between the SWDGE cast path (software queue) and
# the HWDGE fp32 path (hardware queue) so the two DGE rings interleave
# packets across the 16 shared SDMA engines instead of serializing on one
# queue
DUAL_DMA = int(os.environ.get("CLK_DUAL_DMA", "1"))
MASK_BUFS = int(os.environ.get("CLK_MASK_BUFS", "56"))
PF_TAUS = int(os.environ.get("CLK_PF_TAUS", "16"))
AD_BUFS = int(os.environ.get("CLK_AD_BUFS", "6"))
OUT_BUFS = int(os.environ.get("CLK_OUT_BUFS", "8"))
# head slice of cpack loaded by the first (small) input DMA: inb + this
# many wcols, so the first abs doesn't wait for the full wcols load
HEAD_W = int(os.environ.get("CLK_HEAD_W", "32"))

# psum bank counts per group: small leading groups cut time-to-first-DMA,
# small trailing groups shorten the drain tail.  Sums to MB=64.
_RAMP = [1, 1, 2] + [4] * 14 + [2, 1, 1]
assert sum(_RAMP) == MB

LAST_RESULTS = None   # BassKernelResults of the most recent kernel() call
_CACHED_NC = None


def _build_t_matrix() -> np.ndarray:
    t = np.zeros((128, T_TOTAL), dtype=np.float32)
    for d in range(5):
        jl = np.arange(128)[:, None]
        mf = np.arange(T_WIDTHS[d])[None, :] + T_OFFS[d]
        band = ((128 * d + jl - mf) >= 0) & ((128 * d + jl - mf) <= 50)
        t[:, T_COLOFF[d]:T_COLOFF[d + 1]] = band.astype(np.float32)
    return np.concatenate([t, -t], axis=1).astype(ml_dtypes.bfloat16)


def _is_pair(tau: int) -> bool:
    return PM_EVERY > 0 and tau % PM_EVERY == PM_EVERY - 1


def _build_bass() -> bass.Bass:
    nc = bacc.Bacc("TRN2", target_bir_lowering=False, debug=False)

    cpack_d = nc.dram_tensor("cpack", [128, N + JT], F32, kind="ExternalInput").ap()
    tmat_d = nc.dram_tensor("tmat", [128, 2 * T_TOTAL], BF16, kind="ExternalInput").ap()
    out_d = nc.dram_tensor("out", [N, MS], F32, kind="ExternalOutput").ap()

    gw_max = 512 * max(_RAMP)
    with tile.TileContext(nc) as tc:
        with (
            tc.tile_pool(name="consts", bufs=1) as consts,
            tc.tile_pool(name="ad", bufs=AD_BUFS) as ad_pool,
            tc.tile_pool(name="mask", bufs=MASK_BUFS) as mask_pool,
            tc.tile_pool(name="psum", bufs=2, space="PSUM") as psum_pool,
            tc.tile_pool(name="outs", bufs=OUT_BUFS) as out_pool,
        ):
            # split input load: a small head DMA unblocks the first masks,
            # the tail arrives while the ramp groups compute
            cpack = consts.tile([128, N + JT], F32)
            nc.sync.dma_start(cpack[:, 0:N + HEAD_W], cpack_d[:, 0:N + HEAD_W])
            nc.sync.dma_start(cpack[:, N + HEAD_W:], cpack_d[:, N + HEAD_W:])
            inb = cpack[:, 0:N]
            wcols = cpack[:, N:N + JT]
            tmat = consts.tile([128, 2 * T_TOTAL], BF16)
            nc.sync.dma_start(tmat[:], tmat_d[:])

            mask_tiles = {}

            def ensure_mask(tau):
                if tau in mask_tiles:
                    return
                wc = wcols[:, tau:tau + 1]       # per-partition -w
                if _is_pair(tau):
                    mkp = mask_pool.tile([128, N], BF16, name="mk", tag="mk")
                    nc.vector.tensor_scalar(
                        mkp[:], inb[:], wc, float(THRESH),
                        mybir.AluOpType.add, mybir.AluOpType.is_lt,
                    )
                    mkm = mask_pool.tile([128, N], BF16, name="mk", tag="mk")
                    nc.vector.tensor_scalar(
                        mkm[:], inb[:], wc, float(-THRESH),
                        mybir.AluOpType.add, mybir.AluOpType.is_le,
                    )
                    mask_tiles[tau] = ("pair", mkp, mkm)
                else:
                    # ad = |x[n] - w[128*tau + p]|, exact fp32
                    # (wcols holds NEGATED w so it rides the activation bias)
                    ad = ad_pool.tile([128, N], F32)
                    nc.scalar.activation(
                        ad[:], inb[:], mybir.ActivationFunctionType.Abs,
                        bias=wc, scale=1.0,
                    )
                    mk = mask_pool.tile([128, N], BF16, name="mk", tag="mk")
                    nc.vector.tensor_scalar(
                        mk[:], ad[:], float(THRESH), None,
                        mybir.AluOpType.is_lt,
                    )
                    mask_tiles[tau] = ("abs", mk)

            # group schedule: (first bank, nbanks)
            groups = []
            b0 = 0
            for nbk in _RAMP:
                groups.append((b0, nbk))
                b0 += nbk

            th_cnt = 0
            th_act = 0
            pf_ptr = 0
            for gi, (b0, nbk) in enumerate(groups):
                tau_lo, tau_hi = 4 * b0, 4 * (b0 + nbk) + 1
                for t in range(tau_lo, tau_hi):
                    ensure_mask(t)
                pf_ptr = max(pf_ptr, tau_hi)
                pf_target = min(tau_hi + PF_TAUS, JT)
                # drop dead references (slots recycle via the pool)
                for t in list(mask_tiles):
                    if t < tau_lo:
                        del mask_tiles[t]
                gw = 512 * nbk
                for nb in range(NB):
                    ps = psum_pool.tile([128, gw_max], F32)
                    for k in range(nbk):
                        mb = b0 + k
                        mms = []      # (mask_tile, col base in packed tmat, d)
                        for d in range(5):
                            ent = mask_tiles[4 * mb + d]
                            if ent[0] == "abs":
                                mms.append((ent[1], 0, d))
                            else:
                                mms.append((ent[1], 0, d))
                                mms.append((ent[2], T_TOTAL, d))
                        for i, (mt, base, d) in enumerate(mms):
                            nc.tensor.matmul(
                                ps[:, k * 512 + T_OFFS[d]:
                                   k * 512 + T_OFFS[d] + T_WIDTHS[d]],
                                mt[:, nb * 128:(nb + 1) * 128],
                                tmat[:, base + T_COLOFF[d]:
                                     base + T_COLOFF[d] + T_WIDTHS[d]],
                                start=(i == 0), stop=(i == len(mms) - 1),
                            )
                    swdge = CAST_DMA and (not DUAL_DMA or (gi * NB + nb) % 2 == 0)
                    if swdge:
                        ob = out_pool.tile([128, gw_max], BF16,
                                           name="ob", tag="obh", bufs=5)
                    else:
                        ob = out_pool.tile([128, gw_max], F32,
                                           name="ob", tag="obf", bufs=4)
                    use_act = False
                    if gi >= len(groups) - 5:
                        # mask-gen is done by now: ACT is free for most of
                        # the tail drains (also the faster drain engine)
                        th_cnt += 1
                        use_act = th_cnt % 3 != 0
                    elif gi >= 3:   # earliest groups stay on DVE (ACT busy)
                        th_cnt += 1
                        want = (th_cnt * TH_ACT_PCT) // 100
                        if want > th_act:
                            th_act = want
                            use_act = True
                    if use_act:
                        nc.scalar.activation(
                            ob[:, :gw], ps[:, :gw],
                            mybir.ActivationFunctionType.Sign,
                        )
                    else:
                        nc.vector.tensor_scalar(
                            ob[:, :gw], ps[:, :gw], 0.0, None,
                            mybir.AluOpType.is_gt,
                        )
                    dst = out_d[nb * 128:(nb + 1) * 128,
                                b0 * 512:(b0 + nbk) * 512]
                    if swdge:
                        nc.gpsimd.dma_start(dst, ob[:, :gw])
                    else:
                        nc.sync.dma_start(dst, ob[:, :gw])
                    # interleave next-group mask prefetch between drains so
                    # the in-order DVE/ACT queues never stall the out-DMA
                    # behind a 17-tile mask burst
                    for _ in range(3):
                        if pf_ptr < pf_target:
                            ensure_mask(pf_ptr)
                            pf_ptr += 1
    nc.compile()
    return nc


def kernel(input_features: np.ndarray, weight_matrix: np.ndarray) -> np.ndarray:
    global LAST_RESULTS, _CACHED_NC
    flat_in = np.ascontiguousarray(input_features, dtype=np.float32).reshape(-1)
    flat_w = np.ascontiguousarray(weight_matrix, dtype=np.float32).reshape(-1)
    assert flat_in.shape == (N,) and flat_w.shape == (M,)

    # global padded weights: 25 sentinels + w + enough sentinel tail that
    # every core's slice [c*MS, c*MS + JPAD) is in range
    g = np.full(PAD + M + (JPAD - MS - PAD), BIG, dtype=np.float32)
    g[PAD:PAD + M] = flat_w

    inb = np.ascontiguousarray(np.broadcast_to(flat_in[None, :], (128, N)))
    tmat = np.ascontiguousarray(_build_t_matrix())

    in_maps = []
    for c in range(NCORES):
        wc = g[c * MS:c * MS + JPAD]            # [JPAD]
        wcols = np.ascontiguousarray(-wc.reshape(JT, 128).T)  # [128, JT], negated
        cpack = np.ascontiguousarray(
            np.concatenate([inb, wcols], axis=1), dtype=np.float32)
        in_maps.append({"cpack": cpack, "tmat": tmat})

    if _CACHED_NC is None:
        _CACHED_NC = _build_bass()

    LAST_RESULTS = run_bass_kernel_spmd(
        _CACHED_NC, in_maps, core_ids=list(range(NCORES)),
    )
    outs = [r["out"] for r in LAST_RESULTS.results]
    return np.concatenate(outs, axis=1)


if __name__ == "__main__":
    x = np.random.randn(2, 512).astype(np.float32)
    w = np.random.randn(512, 512).astype(np.float32)
    o = kernel(x, w)
    print(o.shape, o.dtype, o.mean())
